# revision 35
# baseline (speedup 1.0000x reference)
"""Multi-head attention Trainium2 kernel (B=4, S=1024, EMB=1024, 16 heads).

Sharding: 8 cores = 4 batches x 2 head-groups. Core c handles batch c//2 and
heads [8*(c%2), 8*(c%2)+8) -- tensor-parallel over heads within a batch.
Each core computes its Q/K/V projections (512 of 1024 e_out columns), full
attention for its 8 heads, and a partial output projection; the two cores
sharing a batch have their partials summed on the host.

Device layouts (per core):
  QT/KT: [e_out, s] transposed projections as SBUF [128p, 4chunk, 1024s]
         (e_out local = chunk*128 + p; head h at chunk h//2, partitions
         64*(h%2)..+64)
  V:     natural [s, e_out] as SBUF [128p, 8st, 8h, 65] -- 64 value dims per
         head plus a constant-ones column, so the P@V matmul's PSUM row 64
         accumulates the softmax denominator for free.
  Scores are computed once per head as S^T=[k,q] (k on partitions, feeding
  P@V directly). The device writes UNNORMALIZED exp(S^T/sqrt(d)) as
  wtsu[h,k,q] plus reciprocal row-sums rsum[h,q]; the host fuses the
  normalize + [k,q]->[q,k] transpose in one einsum.
  All matmuls run in float32r (~1.5e-4 rel err, full PE rate at N=512).
"""

import numpy as np

import concourse.bacc as bacc
import concourse.mybir as mybir
import concourse.tile as tile
from concourse.bass_utils import run_bass_kernel_spmd

B, S, EMB, HEADS, HD = 4, 1024, 1024, 16, 64
SCALE = HD**-0.5
NCORES = 8
HPC = HEADS // 2  # heads per core
ESL = HPC * HD  # e_out slice per core (512)
F32 = mybir.dt.float32
F32R = mybir.dt.float32r
EXP = mybir.ActivationFunctionType.Exp
MULT = mybir.AluOpType.mult

_CACHE = {}


def _build():
    if "nc" in _CACHE:
        return _CACHE["nc"]

    nc = bacc.Bacc("TRN2", target_bir_lowering=False, debug=False, num_devices=NCORES)

    xtq = nc.dram_tensor("xtq", [EMB, S], F32R, kind="ExternalInput")
    xtk = nc.dram_tensor("xtk", [EMB, S], F32R, kind="ExternalInput")
    xtv = nc.dram_tensor("xtv", [EMB, S], F32R, kind="ExternalInput")
    wqt = nc.dram_tensor("wqt", [EMB, ESL], F32R, kind="ExternalInput")
    wkt = nc.dram_tensor("wkt", [EMB, ESL], F32R, kind="ExternalInput")
    wvt = nc.dram_tensor("wvt", [EMB, ESL], F32R, kind="ExternalInput")
    wot = nc.dram_tensor("wot", [ESL, EMB], F32R, kind="ExternalInput")
    bq_d = nc.dram_tensor("bq", [128, 4], F32, kind="ExternalInput")
    bk_d = nc.dram_tensor("bk", [128, 4], F32, kind="ExternalInput")
    bo_d = nc.dram_tensor("bo", [128, 8], F32, kind="ExternalInput")
    vones_d = nc.dram_tensor("vones", [128, 8, HPC], F32R, kind="ExternalInput")
    wtsu_d = nc.dram_tensor("wtsu", [HPC, S, S], F32R, kind="ExternalOutput")
    rsum_d = nc.dram_tensor("rsum", [HPC, S], F32R, kind="ExternalOutput")
    outp_d = nc.dram_tensor("outp", [EMB, S], F32, kind="ExternalOutput")

    with tile.TileContext(nc) as tc, nc.allow_low_precision(
        reason="float32r tiles feed full-rate PE matmuls; accumulation stays fp32"
    ):
        with (
            tc.tile_pool(name="const", bufs=1) as cpool,
            tc.tile_pool(name="qkv", bufs=1) as qkvpool,
            tc.tile_pool(name="wt", bufs=2) as wtpool,
            tc.tile_pool(name="xt", bufs=3) as xtpool,
        ):
            bq_sb = cpool.tile([128, 4], F32)
            bk_sb = cpool.tile([128, 4], F32)
            bo_sb = cpool.tile([128, 8], F32)
            nc.sync.dma_start(bq_sb[:], bq_d.ap())
            nc.sync.dma_start(bk_sb[:], bk_d.ap())
            nc.sync.dma_start(bo_sb[:], bo_d.ap())

            qt_sb = qkvpool.tile([128, 4, S], F32R)
            kt_sb = qkvpool.tile([128, 4, S], F32R)
            v_sb = qkvpool.tile([128, 8, HPC, HD + 1], F32R)
            # constant ones column per head for the in-matmul denominator
            nc.sync.dma_start(
                v_sb[:, :, :, HD : HD + 1],
                vones_d.ap().unsqueeze(3),
            )

            # ---- Phase 1: projections (Q, K chunk-outer; V streamed per
            # s-tile so attention overlaps V's tail) ----
            with tc.tile_pool(name="pjps", bufs=3, space="PSUM") as pjps:
                for pname, xdram, wdram in (
                    ("q", xtq, wqt),
                    ("k", xtk, wkt),
                    ("v", xtv, wvt),
                ):
                    w_sb = wtpool.tile([128, 8, ESL], F32R, tag="wt", name=f"w_{pname}")
                    nc.sync.dma_start(
                        w_sb[:], wdram.ap().rearrange("(kt p) n -> p kt n", p=128)
                    )
                    halves = []
                    for h2 in range(2):
                        x_t = xtpool.tile(
                            [128, 4, S], F32R, tag="xt", name=f"x_{pname}{h2}"
                        )
                        nc.sync.dma_start(
                            x_t[:],
                            xdram.ap()[h2 * 512 : (h2 + 1) * 512, :].rearrange(
                                "(kt p) s -> p kt s", p=128
                            ),
                        )
                        halves.append(x_t)

                    def xslice(kt, lo, hi):
                        return halves[kt // 4][:, kt % 4, lo:hi]

                    if pname == "v":
                        for st in range(8):
                            ps = pjps.tile(
                                [128, 512], F32, tag="pjps", name=f"pj_v_{st}"
                            )
                            for kt in range(8):
                                nc.tensor.matmul(
                                    ps[:],
                                    xslice(kt, st * 128, (st + 1) * 128),
                                    w_sb[:, kt, :],
                                    start=(kt == 0),
                                    stop=(kt == 7),
                                )
                            nc.vector.tensor_copy(
                                v_sb[:, st, :, 0:HD],
                                ps[:].rearrange("p (h e) -> p h e", e=HD),
                            )
                    else:
                        dst = qt_sb if pname == "q" else kt_sb
                        bias = bq_sb if pname == "q" else bk_sb
                        for ch in range(4):
                            for sh in range(2):
                                ps = pjps.tile(
                                    [128, 512], F32, tag="pjps",
                                    name=f"pj_{pname}_{ch}{sh}",
                                )
                                for kt in range(8):
                                    nc.tensor.matmul(
                                        ps[:],
                                        w_sb[:, kt, ch * 128 : (ch + 1) * 128],
                                        xslice(kt, sh * 512, (sh + 1) * 512),
                                        start=(kt == 0),
                                        stop=(kt == 7),
                                    )
                                nc.vector.tensor_scalar_add(
                                    dst[:, ch, sh * 512 : (sh + 1) * 512],
                                    ps[:],
                                    bias[:, ch : ch + 1],
                                )

            # ---- Phase 2: attention, single pass per head ----
            ct_sb = qkvpool.tile([128, 4, S], F32R)
            wo_sb = wtpool.tile([128, 4, EMB], F32R, tag="wt")
            nc.sync.dma_start(
                wo_sb[:], wot.ap().rearrange("(ce p) n -> p ce n", p=128)
            )

            with (
                tc.tile_pool(name="stps", bufs=2, space="PSUM") as stpsum,
                tc.tile_pool(name="pvps", bufs=3, space="PSUM") as pvpsum,
                tc.tile_pool(name="et", bufs=4) as etpool,
                tc.tile_pool(name="small", bufs=3) as smallpool,
                tc.tile_pool(name="bc", bufs=2) as bcpool,
            ):
                # The PE queue is strictly in-order: a PV matmul waiting on
                # its exp would block later (independent) ST matmuls queued
                # behind it. Emit ST two steps ahead of exp/PV so the PE
                # always has ready work in front of any waiting instruction.
                LOOKAHEAD = 1
                steps = [(h, kt) for h in range(HPC) for kt in range(8)]
                st_tiles = {}

                def emit_st(idx):
                    h, kt = steps[idx]
                    hb = 64 * (h % 2)
                    hc = h // 2
                    st_ps = stpsum.tile(
                        [128, 1024], F32, tag="stps", name=f"st_{h}_{kt}"
                    )
                    for qh in range(2):
                        nc.tensor.matmul(
                            st_ps[:, qh * 512 : (qh + 1) * 512],
                            kt_sb[hb : hb + 64, hc, kt * 128 : (kt + 1) * 128],
                            qt_sb[hb : hb + 64, hc, qh * 512 : (qh + 1) * 512],
                            start=True,
                            stop=True,
                            tile_position=(hb, 0),
                        )
                    st_tiles[idx] = st_ps

                def _finish_head(h):
                    hc = h // 2
                    pv0, pv1 = pvs[h]
                    # reciprocal denominators: DVE hop PSUM row 64 -> SBUF
                    # (lanes can't shift partitions), then one DMA splits the
                    # two q-halves onto partitions 0/1
                    sums_row = smallpool.tile(
                        [65, 1024], F32, tag="sumsrow", name=f"sr{h}"
                    )
                    nc.vector.tensor_copy(sums_row[64:65, 0:512], pv0[64:65, :])
                    nc.vector.tensor_copy(sums_row[64:65, 512:1024], pv1[64:65, :])
                    sums2 = smallpool.tile([2, 512], F32, tag="sums2", name=f"sm{h}")
                    nc.gpsimd.dma_start(sums2[:], sums_row[64:65, :])
                    recip2 = smallpool.tile([2, 512], F32R, tag="recip2", name=f"rc{h}")
                    nc.vector.reciprocal(recip2[:], sums2[:])
                    nc.sync.dma_start(rsum_d.ap()[h : h + 1, :], recip2[:, :])
                    bc_sb = bcpool.tile([64, 1024], F32R, tag="bc", name=f"bc{h}")
                    for qh in range(2):
                        nc.sync.dma_start(
                            bc_sb[:, qh * 512 : (qh + 1) * 512],
                            recip2[qh : qh + 1, :]
                            .unsqueeze(1)
                            .to_broadcast([1, 64, 512]),
                        )
                    # matmul dst starts at partition 0; odd heads hop to
                    # partitions 64-127 of ct_sb via a small SBUF DMA
                    if h % 2 == 0:
                        for qh, pv in ((0, pv0), (1, pv1)):
                            nc.vector.tensor_tensor(
                                ct_sb[0:64, hc, qh * 512 : (qh + 1) * 512],
                                pv[0:64, :],
                                bc_sb[0:64, qh * 512 : (qh + 1) * 512],
                                op=MULT,
                            )
                    else:
                        ct_tmp = bcpool.tile(
                            [64, 1024], F32R, tag="cttmp", name=f"ctt{h}"
                        )
                        for qh, pv in ((0, pv0), (1, pv1)):
                            nc.vector.tensor_tensor(
                                ct_tmp[:, qh * 512 : (qh + 1) * 512],
                                pv[0:64, :],
                                bc_sb[0:64, qh * 512 : (qh + 1) * 512],
                                op=MULT,
                            )
                        nc.sync.dma_start(ct_sb[64:128, hc, :], ct_tmp[:])

                pvs = {}
                for h in range(HPC):
                    # rows 0-63: context^T; row 64: softmax denominator
                    pvs[h] = [
                        pvpsum.tile([65, 512], F32, tag="pv", name=f"pv{h}_{qh}")
                        for qh in range(2)
                    ]
                    for kt in range(8):
                        idx = h * 8 + kt
                        if idx == 0:
                            for j in range(LOOKAHEAD + 1):
                                emit_st(j)
                        elif idx + LOOKAHEAD < len(steps):
                            emit_st(idx + LOOKAHEAD)
                        st_ps = st_tiles.pop(idx)
                        et_t = etpool.tile([128, 1024], F32R, tag="et")
                        nc.scalar.activation(et_t[:], st_ps[:], EXP, scale=SCALE)
                        # unnormalized weights out (host normalizes+transposes)
                        nc.gpsimd.dma_start(
                            wtsu_d.ap()[h, kt * 128 : (kt + 1) * 128, :], et_t[:]
                        )
                        for qh, pv in ((0, pvs[h][0]), (1, pvs[h][1])):
                            nc.tensor.matmul(
                                pv[:],
                                v_sb[:, kt, h, :],
                                et_t[:, qh * 512 : (qh + 1) * 512],
                                start=(kt == 0),
                                stop=(kt == 7),
                            )
                    _finish_head(h)

            # ---- Phase 3: output projection (partial; host sums core pairs) ----
            with (
                tc.tile_pool(name="ops", bufs=2, space="PSUM") as oppsum,
                tc.tile_pool(name="osb", bufs=2) as outpool,
            ):
                for ch in range(8):
                    o_sb = outpool.tile([128, 1024], F32, tag="osb")
                    for sh in range(2):
                        o_ps = oppsum.tile([128, 512], F32, tag="ops")
                        for ce in range(4):
                            nc.tensor.matmul(
                                o_ps[:],
                                wo_sb[:, ce, ch * 128 : (ch + 1) * 128],
                                ct_sb[:, ce, sh * 512 : (sh + 1) * 512],
                                start=(ce == 0),
                                stop=(ce == 3),
                            )
                        nc.vector.tensor_scalar_add(
                            o_sb[:, sh * 512 : (sh + 1) * 512],
                            o_ps[:],
                            bo_sb[:, ch : ch + 1],
                        )
                    nc.sync.dma_start(
                        outp_d.ap()[ch * 128 : (ch + 1) * 128, :], o_sb[:]
                    )

    nc.compile()
    _CACHE["nc"] = nc
    return nc


def kernel(**inputs):
    query = np.asarray(inputs["query"], np.float32)
    key = np.asarray(inputs["key"], np.float32)
    value = np.asarray(inputs["value"], np.float32)
    Wq, bq = np.asarray(inputs["Wq"], np.float32), np.asarray(inputs["bq"], np.float32)
    Wk, bk = np.asarray(inputs["Wk"], np.float32), np.asarray(inputs["bk"], np.float32)
    Wv, bv = np.asarray(inputs["Wv"], np.float32), np.asarray(inputs["bv"], np.float32)
    Wo, bo = np.asarray(inputs["Wo"], np.float32), np.asarray(inputs["bo"], np.float32)

    nc = _build()

    in_maps = []
    for c in range(NCORES):
        b, g = divmod(c, 2)
        cols = slice(g * ESL, (g + 1) * ESL)
        # bv folds through the (linear) output projection: W @ (V + bv) adds
        # Wo_c^T @ bv_c per core; bo itself is added by the even core only.
        bo_eff = Wo.T[cols, :].T @ bv[cols]
        if g == 0:
            bo_eff = bo_eff + bo
        in_maps.append(
            {
                "xtq": np.ascontiguousarray(query[b].T),
                "xtk": np.ascontiguousarray(key[b].T),
                "xtv": np.ascontiguousarray(value[b].T),
                "wqt": np.ascontiguousarray(Wq.T[:, cols]),
                "wkt": np.ascontiguousarray(Wk.T[:, cols]),
                "wvt": np.ascontiguousarray(Wv.T[:, cols]),
                "wot": np.ascontiguousarray(Wo.T[cols, :]),
                "bq": np.ascontiguousarray(bq[cols].reshape(4, 128).T),
                "bk": np.ascontiguousarray(bk[cols].reshape(4, 128).T),
                "bo": np.ascontiguousarray(bo_eff.reshape(8, 128).T),
                "vones": np.ones((128, 8, HPC), np.float32),
            }
        )

    res = run_bass_kernel_spmd(nc, in_maps, list(range(NCORES)))

    out = np.empty((B, S, EMB), np.float32)
    wts = np.empty((B, HEADS, S, S), np.float32)
    for c in range(NCORES):
        b, g = divmod(c, 2)
        # normalize + [k,q]->[q,k] transpose fused on host
        wu = res.results[c]["wtsu"]  # [8, k, q] unnormalized exp
        rs = res.results[c]["rsum"]  # [8, q] reciprocal row sums
        np.einsum(
            "hkq,hq->hqk", wu, rs, out=wts[b, g * HPC : (g + 1) * HPC],
            optimize=False,
        )
    for b in range(B):
        out[b] = (res.results[2 * b]["outp"] + res.results[2 * b + 1]["outp"]).T
    return out, wts


# revision 37
# speedup vs baseline: 1.2467x; 1.2467x over previous
"""Multi-head attention Trainium2 kernel (B=4, S=1024, EMB=1024, 16 heads).

Sharding: 8 cores = 4 batches x 2 head-groups. Core c handles batch c//2 and
heads [8*(c%2), 8*(c%2)+8) -- tensor-parallel over heads within a batch.
Each core computes its Q/K/V projections (512 of 1024 e_out columns), full
attention for its 8 heads, and a partial output projection; the two cores
sharing a batch have their partials summed on the host.

Device layouts (per core):
  QT/KT: [e_out, s] transposed projections as SBUF [128p, 4chunk, 1024s]
         (e_out local = chunk*128 + p; head h at chunk h//2, partitions
         64*(h%2)..+64)
  V:     natural [s, e_out] as SBUF [128p, 8st, 8h, 65] -- 64 value dims per
         head plus a constant-ones column, so the P@V matmul's PSUM row 64
         accumulates the softmax denominator for free.
  Scores are computed once per head as S^T=[k,q] (k on partitions, feeding
  P@V directly). The device writes UNNORMALIZED exp(S^T/sqrt(d)) as
  wtsu[h,k,q] plus reciprocal row-sums rsum[h,q]; the host fuses the
  normalize + [k,q]->[q,k] transpose in one einsum.
  All matmuls run in float32r (~1.5e-4 rel err, full PE rate at N=512).
"""

import ml_dtypes
import numpy as np

BF16_NP = ml_dtypes.bfloat16

import concourse.bacc as bacc
import concourse.mybir as mybir
import concourse.tile as tile
from concourse.bass_utils import run_bass_kernel_spmd

B, S, EMB, HEADS, HD = 4, 1024, 1024, 16, 64
SCALE = HD**-0.5
NCORES = 8
HPC = HEADS // 2  # heads per core
ESL = HPC * HD  # e_out slice per core (512)
F32 = mybir.dt.float32
F32R = mybir.dt.float32r
BF16 = mybir.dt.bfloat16
EXP = mybir.ActivationFunctionType.Exp
MULT = mybir.AluOpType.mult

_CACHE = {}


def _build():
    if "nc" in _CACHE:
        return _CACHE["nc"]

    nc = bacc.Bacc("TRN2", target_bir_lowering=False, debug=False, num_devices=NCORES)

    xtq = nc.dram_tensor("xtq", [EMB, S], BF16, kind="ExternalInput")
    xtk = nc.dram_tensor("xtk", [EMB, S], BF16, kind="ExternalInput")
    xtv = nc.dram_tensor("xtv", [EMB, S], BF16, kind="ExternalInput")
    wqt = nc.dram_tensor("wqt", [EMB, ESL], BF16, kind="ExternalInput")
    wkt = nc.dram_tensor("wkt", [EMB, ESL], BF16, kind="ExternalInput")
    wvt = nc.dram_tensor("wvt", [EMB, ESL], BF16, kind="ExternalInput")
    wot = nc.dram_tensor("wot", [ESL, EMB], BF16, kind="ExternalInput")
    bq_d = nc.dram_tensor("bq", [128, 4], F32, kind="ExternalInput")
    bk_d = nc.dram_tensor("bk", [128, 4], F32, kind="ExternalInput")
    bo_d = nc.dram_tensor("bo", [128, 8], F32, kind="ExternalInput")
    vones_d = nc.dram_tensor("vones", [128, 8, HPC], BF16, kind="ExternalInput")
    wtsu_d = nc.dram_tensor("wtsu", [HPC, S, S], BF16, kind="ExternalOutput")
    rsum_d = nc.dram_tensor("rsum", [HPC, S], F32, kind="ExternalOutput")
    outp_d = nc.dram_tensor("outp", [EMB, S], F32, kind="ExternalOutput")

    with tile.TileContext(nc) as tc, nc.allow_low_precision(
        reason="float32r tiles feed full-rate PE matmuls; accumulation stays fp32"
    ):
        with (
            tc.tile_pool(name="const", bufs=1) as cpool,
            tc.tile_pool(name="qkv", bufs=1) as qkvpool,
            tc.tile_pool(name="wt", bufs=2) as wtpool,
            tc.tile_pool(name="xt", bufs=3) as xtpool,
        ):
            bq_sb = cpool.tile([128, 4], F32)
            bk_sb = cpool.tile([128, 4], F32)
            bo_sb = cpool.tile([128, 8], F32)
            nc.sync.dma_start(bq_sb[:], bq_d.ap())
            nc.sync.dma_start(bk_sb[:], bk_d.ap())
            nc.sync.dma_start(bo_sb[:], bo_d.ap())

            qt_sb = qkvpool.tile([128, 4, S], BF16)
            kt_sb = qkvpool.tile([128, 4, S], BF16)
            v_sb = qkvpool.tile([128, 8, HPC, HD + 1], BF16)
            # constant ones column per head for the in-matmul denominator
            nc.sync.dma_start(
                v_sb[:, :, :, HD : HD + 1],
                vones_d.ap().unsqueeze(3),
            )

            # ---- Phase 1: projections (Q, K chunk-outer; V streamed per
            # s-tile so attention overlaps V's tail) ----
            with tc.tile_pool(name="pjps", bufs=3, space="PSUM") as pjps:
                for pname, xdram, wdram in (
                    ("q", xtq, wqt),
                    ("k", xtk, wkt),
                    ("v", xtv, wvt),
                ):
                    w_sb = wtpool.tile([128, 8, ESL], BF16, tag="wt", name=f"w_{pname}")
                    nc.sync.dma_start(
                        w_sb[:], wdram.ap().rearrange("(kt p) n -> p kt n", p=128)
                    )
                    halves = []
                    for h2 in range(2):
                        x_t = xtpool.tile(
                            [128, 4, S], BF16, tag="xt", name=f"x_{pname}{h2}"
                        )
                        nc.sync.dma_start(
                            x_t[:],
                            xdram.ap()[h2 * 512 : (h2 + 1) * 512, :].rearrange(
                                "(kt p) s -> p kt s", p=128
                            ),
                        )
                        halves.append(x_t)

                    def xslice(kt, lo, hi):
                        return halves[kt // 4][:, kt % 4, lo:hi]

                    if pname == "v":
                        for st in range(8):
                            ps = pjps.tile(
                                [128, 512], F32, tag="pjps", name=f"pj_v_{st}"
                            )
                            for kt in range(8):
                                nc.tensor.matmul(
                                    ps[:],
                                    xslice(kt, st * 128, (st + 1) * 128),
                                    w_sb[:, kt, :],
                                    start=(kt == 0),
                                    stop=(kt == 7),
                                )
                            nc.vector.tensor_copy(
                                v_sb[:, st, :, 0:HD],
                                ps[:].rearrange("p (h e) -> p h e", e=HD),
                            )
                    else:
                        dst = qt_sb if pname == "q" else kt_sb
                        bias = bq_sb if pname == "q" else bk_sb
                        for ch in range(4):
                            for sh in range(2):
                                ps = pjps.tile(
                                    [128, 512], F32, tag="pjps",
                                    name=f"pj_{pname}_{ch}{sh}",
                                )
                                for kt in range(8):
                                    nc.tensor.matmul(
                                        ps[:],
                                        w_sb[:, kt, ch * 128 : (ch + 1) * 128],
                                        xslice(kt, sh * 512, (sh + 1) * 512),
                                        start=(kt == 0),
                                        stop=(kt == 7),
                                    )
                                nc.vector.tensor_scalar_add(
                                    dst[:, ch, sh * 512 : (sh + 1) * 512],
                                    ps[:],
                                    bias[:, ch : ch + 1],
                                )

            # ---- Phase 2: attention, single pass per head ----
            ct_sb = qkvpool.tile([128, 4, S], BF16)
            wo_sb = wtpool.tile([128, 4, EMB], BF16, tag="wt")
            nc.sync.dma_start(
                wo_sb[:], wot.ap().rearrange("(ce p) n -> p ce n", p=128)
            )

            with (
                tc.tile_pool(name="stps", bufs=2, space="PSUM") as stpsum,
                tc.tile_pool(name="pvps", bufs=3, space="PSUM") as pvpsum,
                tc.tile_pool(name="et", bufs=4) as etpool,
                tc.tile_pool(name="small", bufs=3) as smallpool,
                tc.tile_pool(name="bc", bufs=2) as bcpool,
            ):
                # The PE queue is strictly in-order: a PV matmul waiting on
                # its exp would block later (independent) ST matmuls queued
                # behind it. Emit ST two steps ahead of exp/PV so the PE
                # always has ready work in front of any waiting instruction.
                LOOKAHEAD = 1
                steps = [(h, kt) for h in range(HPC) for kt in range(8)]
                st_tiles = {}

                def emit_st(idx):
                    h, kt = steps[idx]
                    hb = 64 * (h % 2)
                    hc = h // 2
                    st_ps = stpsum.tile(
                        [128, 1024], F32, tag="stps", name=f"st_{h}_{kt}"
                    )
                    for qh in range(2):
                        nc.tensor.matmul(
                            st_ps[:, qh * 512 : (qh + 1) * 512],
                            kt_sb[hb : hb + 64, hc, kt * 128 : (kt + 1) * 128],
                            qt_sb[hb : hb + 64, hc, qh * 512 : (qh + 1) * 512],
                            start=True,
                            stop=True,
                            tile_position=(hb, 0),
                        )
                    st_tiles[idx] = st_ps

                def _finish_head(h):
                    hc = h // 2
                    pv0, pv1 = pvs[h]
                    # reciprocal denominators: DVE hop PSUM row 64 -> SBUF
                    # (lanes can't shift partitions), then one DMA splits the
                    # two q-halves onto partitions 0/1
                    sums_row = smallpool.tile(
                        [65, 1024], F32, tag="sumsrow", name=f"sr{h}"
                    )
                    nc.vector.tensor_copy(sums_row[64:65, 0:512], pv0[64:65, :])
                    nc.vector.tensor_copy(sums_row[64:65, 512:1024], pv1[64:65, :])
                    sums2 = smallpool.tile([2, 512], F32, tag="sums2", name=f"sm{h}")
                    nc.gpsimd.dma_start(sums2[:], sums_row[64:65, :])
                    recip2 = smallpool.tile([2, 512], F32, tag="recip2", name=f"rc{h}")
                    nc.vector.reciprocal(recip2[:], sums2[:])
                    nc.sync.dma_start(rsum_d.ap()[h : h + 1, :], recip2[:, :])
                    bc_sb = bcpool.tile([64, 1024], F32, tag="bc", name=f"bc{h}")
                    for qh in range(2):
                        nc.sync.dma_start(
                            bc_sb[:, qh * 512 : (qh + 1) * 512],
                            recip2[qh : qh + 1, :]
                            .unsqueeze(1)
                            .to_broadcast([1, 64, 512]),
                        )
                    # matmul dst starts at partition 0; odd heads hop to
                    # partitions 64-127 of ct_sb via a small SBUF DMA
                    if h % 2 == 0:
                        for qh, pv in ((0, pv0), (1, pv1)):
                            nc.vector.tensor_tensor(
                                ct_sb[0:64, hc, qh * 512 : (qh + 1) * 512],
                                pv[0:64, :],
                                bc_sb[0:64, qh * 512 : (qh + 1) * 512],
                                op=MULT,
                            )
                    else:
                        ct_tmp = bcpool.tile(
                            [64, 1024], BF16, tag="cttmp", name=f"ctt{h}"
                        )
                        for qh, pv in ((0, pv0), (1, pv1)):
                            nc.vector.tensor_tensor(
                                ct_tmp[:, qh * 512 : (qh + 1) * 512],
                                pv[0:64, :],
                                bc_sb[0:64, qh * 512 : (qh + 1) * 512],
                                op=MULT,
                            )
                        nc.sync.dma_start(ct_sb[64:128, hc, :], ct_tmp[:])

                pvs = {}
                for h in range(HPC):
                    # rows 0-63: context^T; row 64: softmax denominator
                    pvs[h] = [
                        pvpsum.tile([65, 512], F32, tag="pv", name=f"pv{h}_{qh}")
                        for qh in range(2)
                    ]
                    for kt in range(8):
                        idx = h * 8 + kt
                        if idx == 0:
                            for j in range(LOOKAHEAD + 1):
                                emit_st(j)
                        elif idx + LOOKAHEAD < len(steps):
                            emit_st(idx + LOOKAHEAD)
                        st_ps = st_tiles.pop(idx)
                        et_t = etpool.tile([128, 1024], BF16, tag="et")
                        nc.scalar.activation(et_t[:], st_ps[:], EXP, scale=SCALE)
                        # unnormalized weights out (host normalizes+transposes)
                        nc.gpsimd.dma_start(
                            wtsu_d.ap()[h, kt * 128 : (kt + 1) * 128, :], et_t[:]
                        )
                        for qh, pv in ((0, pvs[h][0]), (1, pvs[h][1])):
                            nc.tensor.matmul(
                                pv[:],
                                v_sb[:, kt, h, :],
                                et_t[:, qh * 512 : (qh + 1) * 512],
                                start=(kt == 0),
                                stop=(kt == 7),
                            )
                    _finish_head(h)

            # ---- Phase 3: output projection (partial; host sums core pairs) ----
            with (
                tc.tile_pool(name="ops", bufs=2, space="PSUM") as oppsum,
                tc.tile_pool(name="osb", bufs=2) as outpool,
            ):
                for ch in range(8):
                    o_sb = outpool.tile([128, 1024], F32, tag="osb")
                    for sh in range(2):
                        o_ps = oppsum.tile([128, 512], F32, tag="ops")
                        for ce in range(4):
                            nc.tensor.matmul(
                                o_ps[:],
                                wo_sb[:, ce, ch * 128 : (ch + 1) * 128],
                                ct_sb[:, ce, sh * 512 : (sh + 1) * 512],
                                start=(ce == 0),
                                stop=(ce == 3),
                            )
                        nc.vector.tensor_scalar_add(
                            o_sb[:, sh * 512 : (sh + 1) * 512],
                            o_ps[:],
                            bo_sb[:, ch : ch + 1],
                        )
                    nc.sync.dma_start(
                        outp_d.ap()[ch * 128 : (ch + 1) * 128, :], o_sb[:]
                    )

    nc.compile()
    _CACHE["nc"] = nc
    return nc


def kernel(**inputs):
    query = np.asarray(inputs["query"], np.float32)
    key = np.asarray(inputs["key"], np.float32)
    value = np.asarray(inputs["value"], np.float32)
    Wq, bq = np.asarray(inputs["Wq"], np.float32), np.asarray(inputs["bq"], np.float32)
    Wk, bk = np.asarray(inputs["Wk"], np.float32), np.asarray(inputs["bk"], np.float32)
    Wv, bv = np.asarray(inputs["Wv"], np.float32), np.asarray(inputs["bv"], np.float32)
    Wo, bo = np.asarray(inputs["Wo"], np.float32), np.asarray(inputs["bo"], np.float32)

    nc = _build()

    in_maps = []
    for c in range(NCORES):
        b, g = divmod(c, 2)
        cols = slice(g * ESL, (g + 1) * ESL)
        # bv folds through the (linear) output projection: W @ (V + bv) adds
        # Wo_c^T @ bv_c per core; bo itself is added by the even core only.
        bo_eff = Wo.T[cols, :].T @ bv[cols]
        if g == 0:
            bo_eff = bo_eff + bo
        in_maps.append(
            {
                "xtq": np.ascontiguousarray(query[b].T).astype(BF16_NP),
                "xtk": np.ascontiguousarray(key[b].T).astype(BF16_NP),
                "xtv": np.ascontiguousarray(value[b].T).astype(BF16_NP),
                "wqt": np.ascontiguousarray(Wq.T[:, cols]).astype(BF16_NP),
                "wkt": np.ascontiguousarray(Wk.T[:, cols]).astype(BF16_NP),
                "wvt": np.ascontiguousarray(Wv.T[:, cols]).astype(BF16_NP),
                "wot": np.ascontiguousarray(Wo.T[cols, :]).astype(BF16_NP),
                "bq": np.ascontiguousarray(bq[cols].reshape(4, 128).T),
                "bk": np.ascontiguousarray(bk[cols].reshape(4, 128).T),
                "bo": np.ascontiguousarray(bo_eff.reshape(8, 128).T),
                "vones": np.ones((128, 8, HPC), BF16_NP),
            }
        )

    res = run_bass_kernel_spmd(nc, in_maps, list(range(NCORES)))

    out = np.empty((B, S, EMB), np.float32)
    wts = np.empty((B, HEADS, S, S), np.float32)
    for c in range(NCORES):
        b, g = divmod(c, 2)
        # normalize + [k,q]->[q,k] transpose fused on host
        wu = res.results[c]["wtsu"].astype(np.float32)  # [8,k,q] unnorm exp
        rs = res.results[c]["rsum"]  # [8, q] reciprocal row sums
        np.einsum(
            "hkq,hq->hqk", wu, rs, out=wts[b, g * HPC : (g + 1) * HPC],
            optimize=False,
        )
    for b in range(B):
        out[b] = (res.results[2 * b]["outp"] + res.results[2 * b + 1]["outp"]).T
    return out, wts


# revision 38
# speedup vs baseline: 1.2771x; 1.0243x over previous
"""Multi-head attention Trainium2 kernel (B=4, S=1024, EMB=1024, 16 heads).

Sharding: 8 cores = 4 batches x 2 head-groups. Core c handles batch c//2 and
heads [8*(c%2), 8*(c%2)+8) -- tensor-parallel over heads within a batch.
Each core computes its Q/K/V projections (512 of 1024 e_out columns), full
attention for its 8 heads, and a partial output projection; the two cores
sharing a batch have their partials summed on the host.

Device layouts (per core):
  QT/KT: [e_out, s] transposed projections as SBUF [128p, 4chunk, 1024s]
         (e_out local = chunk*128 + p; head h at chunk h//2, partitions
         64*(h%2)..+64)
  V:     natural [s, e_out] as SBUF [128p, 8st, 8h, 65] -- 64 value dims per
         head plus a constant-ones column, so the P@V matmul's PSUM row 64
         accumulates the softmax denominator for free.
  Scores are computed once per head as S^T=[k,q] (k on partitions, feeding
  P@V directly). The device writes UNNORMALIZED exp(S^T/sqrt(d)) as
  wtsu[h,k,q] plus reciprocal row-sums rsum[h,q]; the host fuses the
  normalize + [k,q]->[q,k] transpose in one einsum.
  All matmuls run in float32r (~1.5e-4 rel err, full PE rate at N=512).
"""

import ml_dtypes
import numpy as np

BF16_NP = ml_dtypes.bfloat16

import concourse.bacc as bacc
import concourse.mybir as mybir
import concourse.tile as tile
from concourse.bass_utils import run_bass_kernel_spmd

B, S, EMB, HEADS, HD = 4, 1024, 1024, 16, 64
SCALE = HD**-0.5
NCORES = 8
HPC = HEADS // 2  # heads per core
ESL = HPC * HD  # e_out slice per core (512)
F32 = mybir.dt.float32
F32R = mybir.dt.float32r
BF16 = mybir.dt.bfloat16
EXP = mybir.ActivationFunctionType.Exp
MULT = mybir.AluOpType.mult

_CACHE = {}


def _build():
    if "nc" in _CACHE:
        return _CACHE["nc"]

    nc = bacc.Bacc("TRN2", target_bir_lowering=False, debug=False, num_devices=NCORES)

    xtq = nc.dram_tensor("xtq", [EMB, S], BF16, kind="ExternalInput")
    xtk = nc.dram_tensor("xtk", [EMB, S], BF16, kind="ExternalInput")
    xtv = nc.dram_tensor("xtv", [EMB, S], BF16, kind="ExternalInput")
    wqt = nc.dram_tensor("wqt", [EMB, ESL], BF16, kind="ExternalInput")
    wkt = nc.dram_tensor("wkt", [EMB, ESL], BF16, kind="ExternalInput")
    wvt = nc.dram_tensor("wvt", [EMB, ESL], BF16, kind="ExternalInput")
    wot = nc.dram_tensor("wot", [ESL, EMB], BF16, kind="ExternalInput")
    bq_d = nc.dram_tensor("bq", [128, 4], F32, kind="ExternalInput")
    bk_d = nc.dram_tensor("bk", [128, 4], F32, kind="ExternalInput")
    bo_d = nc.dram_tensor("bo", [128, 8], F32, kind="ExternalInput")
    vones_d = nc.dram_tensor("vones", [128, 8, HPC], BF16, kind="ExternalInput")
    wtsu_d = nc.dram_tensor("wtsu", [HPC, S, S], BF16, kind="ExternalOutput")
    rsum_d = nc.dram_tensor("rsum", [HPC, S], F32, kind="ExternalOutput")
    outp_d = nc.dram_tensor("outp", [EMB, S], F32, kind="ExternalOutput")

    with tile.TileContext(nc) as tc, nc.allow_low_precision(
        reason="float32r tiles feed full-rate PE matmuls; accumulation stays fp32"
    ):
        with (
            tc.tile_pool(name="const", bufs=1) as cpool,
            tc.tile_pool(name="qkv", bufs=1) as qkvpool,
            tc.tile_pool(name="wt", bufs=2) as wtpool,
            tc.tile_pool(name="xt", bufs=3) as xtpool,
        ):
            bq_sb = cpool.tile([128, 4], F32)
            bk_sb = cpool.tile([128, 4], F32)
            bo_sb = cpool.tile([128, 8], F32)
            nc.sync.dma_start(bq_sb[:], bq_d.ap())
            nc.sync.dma_start(bk_sb[:], bk_d.ap())
            nc.sync.dma_start(bo_sb[:], bo_d.ap())

            qt_sb = qkvpool.tile([128, 4, S], BF16)
            kt_sb = qkvpool.tile([128, 4, S], BF16)
            v_sb = qkvpool.tile([128, 8, HPC, HD + 1], BF16)
            # constant ones column per head for the in-matmul denominator
            nc.sync.dma_start(
                v_sb[:, :, :, HD : HD + 1],
                vones_d.ap().unsqueeze(3),
            )

            # ---- Phase 1: projections (Q, K chunk-outer; V streamed per
            # s-tile so attention overlaps V's tail) ----
            with tc.tile_pool(name="pjps", bufs=3, space="PSUM") as pjps:
                for pname, xdram, wdram in (
                    ("q", xtq, wqt),
                    ("k", xtk, wkt),
                    ("v", xtv, wvt),
                ):
                    w_sb = wtpool.tile([128, 8, ESL], BF16, tag="wt", name=f"w_{pname}")
                    nc.sync.dma_start(
                        w_sb[:], wdram.ap().rearrange("(kt p) n -> p kt n", p=128)
                    )
                    halves = []
                    for h2 in range(2):
                        x_t = xtpool.tile(
                            [128, 4, S], BF16, tag="xt", name=f"x_{pname}{h2}"
                        )
                        nc.sync.dma_start(
                            x_t[:],
                            xdram.ap()[h2 * 512 : (h2 + 1) * 512, :].rearrange(
                                "(kt p) s -> p kt s", p=128
                            ),
                        )
                        halves.append(x_t)

                    def xslice(kt, lo, hi):
                        return halves[kt // 4][:, kt % 4, lo:hi]

                    if pname == "v":
                        for st in range(8):
                            ps = pjps.tile(
                                [128, 512], F32, tag="pjps", name=f"pj_v_{st}"
                            )
                            for kt in range(8):
                                nc.tensor.matmul(
                                    ps[:],
                                    xslice(kt, st * 128, (st + 1) * 128),
                                    w_sb[:, kt, :],
                                    start=(kt == 0),
                                    stop=(kt == 7),
                                )
                            nc.vector.tensor_copy(
                                v_sb[:, st, :, 0:HD],
                                ps[:].rearrange("p (h e) -> p h e", e=HD),
                            )
                    else:
                        dst = qt_sb if pname == "q" else kt_sb
                        bias = bq_sb if pname == "q" else bk_sb
                        for ch in range(4):
                            for sh in range(2):
                                ps = pjps.tile(
                                    [128, 512], F32, tag="pjps",
                                    name=f"pj_{pname}_{ch}{sh}",
                                )
                                for kt in range(8):
                                    nc.tensor.matmul(
                                        ps[:],
                                        w_sb[:, kt, ch * 128 : (ch + 1) * 128],
                                        xslice(kt, sh * 512, (sh + 1) * 512),
                                        start=(kt == 0),
                                        stop=(kt == 7),
                                    )
                                nc.vector.tensor_scalar_add(
                                    dst[:, ch, sh * 512 : (sh + 1) * 512],
                                    ps[:],
                                    bias[:, ch : ch + 1],
                                )

            # ---- Phase 2: attention, single pass per head ----
            ct_sb = qkvpool.tile([128, 4, S], BF16)
            wo_sb = wtpool.tile([128, 4, EMB], BF16, tag="wt")
            nc.sync.dma_start(
                wo_sb[:], wot.ap().rearrange("(ce p) n -> p ce n", p=128)
            )

            with (
                tc.tile_pool(name="stps", bufs=2, space="PSUM") as stpsum,
                tc.tile_pool(name="pvps", bufs=4, space="PSUM") as pvpsum,
                tc.tile_pool(name="et", bufs=4) as etpool,
                tc.tile_pool(name="small", bufs=3) as smallpool,
                tc.tile_pool(name="bc", bufs=2) as bcpool,
            ):
                # The PE queue is strictly in-order: a PV matmul waiting on
                # its exp would block later (independent) ST matmuls queued
                # behind it. Emit ST two steps ahead of exp/PV so the PE
                # always has ready work in front of any waiting instruction.
                LOOKAHEAD = 1
                steps = [(h, kt) for h in range(HPC) for kt in range(8)]
                st_tiles = {}

                def emit_st(idx):
                    h, kt = steps[idx]
                    hb = 64 * (h % 2)
                    hc = h // 2
                    st_ps = stpsum.tile(
                        [128, 1024], F32, tag="stps", name=f"st_{h}_{kt}"
                    )
                    for qh in range(2):
                        nc.tensor.matmul(
                            st_ps[:, qh * 512 : (qh + 1) * 512],
                            kt_sb[hb : hb + 64, hc, kt * 128 : (kt + 1) * 128],
                            qt_sb[hb : hb + 64, hc, qh * 512 : (qh + 1) * 512],
                            start=True,
                            stop=True,
                            tile_position=(hb, 0),
                        )
                    st_tiles[idx] = st_ps

                def _finish_head(h):
                    hc = h // 2
                    pv0, pv1 = pvs[h]
                    # reciprocal denominators: DVE hop PSUM row 64 -> SBUF
                    # (lanes can't shift partitions), then one DMA splits the
                    # two q-halves onto partitions 0/1
                    sums_row = smallpool.tile(
                        [65, 1024], F32, tag="sumsrow", name=f"sr{h}"
                    )
                    nc.vector.tensor_copy(sums_row[64:65, 0:512], pv0[64:65, :])
                    nc.vector.tensor_copy(sums_row[64:65, 512:1024], pv1[64:65, :])
                    sums2 = smallpool.tile([2, 512], F32, tag="sums2", name=f"sm{h}")
                    nc.gpsimd.dma_start(sums2[:], sums_row[64:65, :])
                    recip2 = smallpool.tile([2, 512], F32, tag="recip2", name=f"rc{h}")
                    nc.vector.reciprocal(recip2[:], sums2[:])
                    nc.sync.dma_start(rsum_d.ap()[h : h + 1, :], recip2[:, :])
                    bc_sb = bcpool.tile([64, 1024], F32, tag="bc", name=f"bc{h}")
                    for qh in range(2):
                        nc.sync.dma_start(
                            bc_sb[:, qh * 512 : (qh + 1) * 512],
                            recip2[qh : qh + 1, :]
                            .unsqueeze(1)
                            .to_broadcast([1, 64, 512]),
                        )
                    # matmul dst starts at partition 0; odd heads hop to
                    # partitions 64-127 of ct_sb via a small SBUF DMA
                    if h % 2 == 0:
                        for qh, pv in ((0, pv0), (1, pv1)):
                            nc.vector.tensor_tensor(
                                ct_sb[0:64, hc, qh * 512 : (qh + 1) * 512],
                                pv[0:64, :],
                                bc_sb[0:64, qh * 512 : (qh + 1) * 512],
                                op=MULT,
                            )
                    else:
                        ct_tmp = bcpool.tile(
                            [64, 1024], BF16, tag="cttmp", name=f"ctt{h}"
                        )
                        for qh, pv in ((0, pv0), (1, pv1)):
                            nc.vector.tensor_tensor(
                                ct_tmp[:, qh * 512 : (qh + 1) * 512],
                                pv[0:64, :],
                                bc_sb[0:64, qh * 512 : (qh + 1) * 512],
                                op=MULT,
                            )
                        nc.sync.dma_start(ct_sb[64:128, hc, :], ct_tmp[:])

                pvs = {}
                for h in range(HPC):
                    # rows 0-63: context^T; row 64: softmax denominator
                    pvs[h] = [
                        pvpsum.tile([65, 512], F32, tag="pv", name=f"pv{h}_{qh}")
                        for qh in range(2)
                    ]
                    for kt in range(8):
                        idx = h * 8 + kt
                        if idx == 0:
                            for j in range(LOOKAHEAD + 1):
                                emit_st(j)
                        elif idx + LOOKAHEAD < len(steps):
                            emit_st(idx + LOOKAHEAD)
                        st_ps = st_tiles.pop(idx)
                        et_t = etpool.tile([128, 1024], BF16, tag="et")
                        nc.scalar.activation(et_t[:], st_ps[:], EXP, scale=SCALE)
                        # unnormalized weights out (host normalizes+transposes)
                        nc.gpsimd.dma_start(
                            wtsu_d.ap()[h, kt * 128 : (kt + 1) * 128, :], et_t[:]
                        )
                        for qh, pv in ((0, pvs[h][0]), (1, pvs[h][1])):
                            nc.tensor.matmul(
                                pv[:],
                                v_sb[:, kt, h, :],
                                et_t[:, qh * 512 : (qh + 1) * 512],
                                start=(kt == 0),
                                stop=(kt == 7),
                            )
                    _finish_head(h)

            # ---- Phase 3: output projection (partial; host sums core pairs) ----
            with (
                tc.tile_pool(name="ops", bufs=2, space="PSUM") as oppsum,
                tc.tile_pool(name="osb", bufs=2) as outpool,
            ):
                for ch in range(8):
                    o_sb = outpool.tile([128, 1024], F32, tag="osb")
                    for sh in range(2):
                        o_ps = oppsum.tile([128, 512], F32, tag="ops")
                        for ce in range(4):
                            nc.tensor.matmul(
                                o_ps[:],
                                wo_sb[:, ce, ch * 128 : (ch + 1) * 128],
                                ct_sb[:, ce, sh * 512 : (sh + 1) * 512],
                                start=(ce == 0),
                                stop=(ce == 3),
                            )
                        nc.vector.tensor_scalar_add(
                            o_sb[:, sh * 512 : (sh + 1) * 512],
                            o_ps[:],
                            bo_sb[:, ch : ch + 1],
                        )
                    nc.sync.dma_start(
                        outp_d.ap()[ch * 128 : (ch + 1) * 128, :], o_sb[:]
                    )

    nc.compile()
    _CACHE["nc"] = nc
    return nc


def kernel(**inputs):
    query = np.asarray(inputs["query"], np.float32)
    key = np.asarray(inputs["key"], np.float32)
    value = np.asarray(inputs["value"], np.float32)
    Wq, bq = np.asarray(inputs["Wq"], np.float32), np.asarray(inputs["bq"], np.float32)
    Wk, bk = np.asarray(inputs["Wk"], np.float32), np.asarray(inputs["bk"], np.float32)
    Wv, bv = np.asarray(inputs["Wv"], np.float32), np.asarray(inputs["bv"], np.float32)
    Wo, bo = np.asarray(inputs["Wo"], np.float32), np.asarray(inputs["bo"], np.float32)

    nc = _build()

    in_maps = []
    for c in range(NCORES):
        b, g = divmod(c, 2)
        cols = slice(g * ESL, (g + 1) * ESL)
        # bv folds through the (linear) output projection: W @ (V + bv) adds
        # Wo_c^T @ bv_c per core; bo itself is added by the even core only.
        bo_eff = Wo.T[cols, :].T @ bv[cols]
        if g == 0:
            bo_eff = bo_eff + bo
        in_maps.append(
            {
                "xtq": np.ascontiguousarray(query[b].T).astype(BF16_NP),
                "xtk": np.ascontiguousarray(key[b].T).astype(BF16_NP),
                "xtv": np.ascontiguousarray(value[b].T).astype(BF16_NP),
                "wqt": np.ascontiguousarray(Wq.T[:, cols]).astype(BF16_NP),
                "wkt": np.ascontiguousarray(Wk.T[:, cols]).astype(BF16_NP),
                "wvt": np.ascontiguousarray(Wv.T[:, cols]).astype(BF16_NP),
                "wot": np.ascontiguousarray(Wo.T[cols, :]).astype(BF16_NP),
                "bq": np.ascontiguousarray(bq[cols].reshape(4, 128).T),
                "bk": np.ascontiguousarray(bk[cols].reshape(4, 128).T),
                "bo": np.ascontiguousarray(bo_eff.reshape(8, 128).T),
                "vones": np.ones((128, 8, HPC), BF16_NP),
            }
        )

    res = run_bass_kernel_spmd(nc, in_maps, list(range(NCORES)))

    out = np.empty((B, S, EMB), np.float32)
    wts = np.empty((B, HEADS, S, S), np.float32)
    for c in range(NCORES):
        b, g = divmod(c, 2)
        # normalize + [k,q]->[q,k] transpose fused on host
        wu = res.results[c]["wtsu"].astype(np.float32)  # [8,k,q] unnorm exp
        rs = res.results[c]["rsum"]  # [8, q] reciprocal row sums
        np.einsum(
            "hkq,hq->hqk", wu, rs, out=wts[b, g * HPC : (g + 1) * HPC],
            optimize=False,
        )
    for b in range(B):
        out[b] = (res.results[2 * b]["outp"] + res.results[2 * b + 1]["outp"]).T
    return out, wts


# revision 40
# speedup vs baseline: 1.3241x; 1.0368x over previous
"""Multi-head attention Trainium2 kernel (B=4, S=1024, EMB=1024, 16 heads).

Sharding: 8 cores = 4 batches x 2 head-groups. Core c handles batch c//2 and
heads [8*(c%2), 8*(c%2)+8) -- tensor-parallel over heads within a batch.
Each core computes its Q/K/V projections (512 of 1024 e_out columns), full
attention for its 8 heads, and a partial output projection; the two cores
sharing a batch have their partials summed on the host.

Device layouts (per core):
  QT/KT: [e_out, s] transposed projections as SBUF [128p, 4chunk, 1024s]
         (e_out local = chunk*128 + p; head h at chunk h//2, partitions
         64*(h%2)..+64)
  V:     natural [s, e_out] as SBUF [128p, 8st, 8h, 65] -- 64 value dims per
         head plus a constant-ones column, so the P@V matmul's PSUM row 64
         accumulates the softmax denominator for free.
  Scores are computed once per head as S^T=[k,q] (k on partitions, feeding
  P@V directly). The device writes UNNORMALIZED exp(S^T/sqrt(d)) as
  wtsu[h,k,q] plus reciprocal row-sums rsum[h,q]; the host fuses the
  normalize + [k,q]->[q,k] transpose in one einsum.
  All matmuls run in float32r (~1.5e-4 rel err, full PE rate at N=512).
"""

import ml_dtypes
import numpy as np

BF16_NP = ml_dtypes.bfloat16

import concourse.bacc as bacc
import concourse.mybir as mybir
import concourse.tile as tile
from concourse.bass_utils import run_bass_kernel_spmd

B, S, EMB, HEADS, HD = 4, 1024, 1024, 16, 64
SCALE = HD**-0.5
NCORES = 8
HPC = HEADS // 2  # heads per core
ESL = HPC * HD  # e_out slice per core (512)
F32 = mybir.dt.float32
F32R = mybir.dt.float32r
BF16 = mybir.dt.bfloat16
EXP = mybir.ActivationFunctionType.Exp
MULT = mybir.AluOpType.mult

_CACHE = {}


def _build():
    if "nc" in _CACHE:
        return _CACHE["nc"]

    nc = bacc.Bacc("TRN2", target_bir_lowering=False, debug=False, num_devices=NCORES)

    xtq = nc.dram_tensor("xtq", [EMB, S], BF16, kind="ExternalInput")
    xtk = nc.dram_tensor("xtk", [EMB, S], BF16, kind="ExternalInput")
    xtv = nc.dram_tensor("xtv", [EMB, S], BF16, kind="ExternalInput")
    wqt = nc.dram_tensor("wqt", [EMB, ESL], BF16, kind="ExternalInput")
    wkt = nc.dram_tensor("wkt", [EMB, ESL], BF16, kind="ExternalInput")
    wvt = nc.dram_tensor("wvt", [EMB, ESL], BF16, kind="ExternalInput")
    wot = nc.dram_tensor("wot", [ESL, EMB], BF16, kind="ExternalInput")
    bq_d = nc.dram_tensor("bq", [128, 4], F32, kind="ExternalInput")
    bk_d = nc.dram_tensor("bk", [128, 4], F32, kind="ExternalInput")
    bo_d = nc.dram_tensor("bo", [128, 8], F32, kind="ExternalInput")
    vones_d = nc.dram_tensor("vones", [128, 8, HPC], F32R, kind="ExternalInput")
    wtsu_d = nc.dram_tensor("wtsu", [HPC, S, S], F32R, kind="ExternalOutput")
    rsum_d = nc.dram_tensor("rsum", [HPC, S], F32, kind="ExternalOutput")
    outp_d = nc.dram_tensor("outp", [EMB, S], F32, kind="ExternalOutput")

    with tile.TileContext(nc) as tc, nc.allow_low_precision(
        reason="float32r tiles feed full-rate PE matmuls; accumulation stays fp32"
    ):
        with (
            tc.tile_pool(name="const", bufs=1) as cpool,
            tc.tile_pool(name="qkv", bufs=1) as qkvpool,
            tc.tile_pool(name="wt", bufs=2) as wtpool,
            tc.tile_pool(name="xt", bufs=4) as xtpool,
        ):
            bq_sb = cpool.tile([128, 4], F32)
            bk_sb = cpool.tile([128, 4], F32)
            bo_sb = cpool.tile([128, 8], F32)
            nc.sync.dma_start(bq_sb[:], bq_d.ap())
            nc.sync.dma_start(bk_sb[:], bk_d.ap())
            nc.sync.dma_start(bo_sb[:], bo_d.ap())

            qt_sb = qkvpool.tile([128, 4, S], F32R)
            kt_sb = qkvpool.tile([128, 4, S], F32R)
            v_sb = qkvpool.tile([128, 8, HPC, HD + 1], F32R)
            # constant ones column per head for the in-matmul denominator
            nc.sync.dma_start(
                v_sb[:, :, :, HD : HD + 1],
                vones_d.ap().unsqueeze(3),
            )

            # ---- Phase 1: projections (Q, K chunk-outer; V streamed per
            # s-tile so attention overlaps V's tail) ----
            with tc.tile_pool(name="pjps", bufs=3, space="PSUM") as pjps:
                w_q = wtpool.tile([128, 8, ESL], BF16, tag="wt", name="w_q")
                nc.sync.dma_start(
                    w_q[:], wqt.ap().rearrange("(kt p) n -> p kt n", p=128)
                )
                w_k = wtpool.tile([128, 8, ESL], BF16, tag="wt", name="w_k")
                nc.sync.dma_start(
                    w_k[:], wkt.ap().rearrange("(kt p) n -> p kt n", p=128)
                )
                xts = {}
                for pn, xdram in (("q", xtq), ("k", xtk)):
                    for h2 in range(2):
                        x_t = xtpool.tile(
                            [128, 4, S], BF16, tag="xt", name=f"x_{pn}{h2}"
                        )
                        nc.sync.dma_start(
                            x_t[:],
                            xdram.ap()[h2 * 512 : (h2 + 1) * 512, :].rearrange(
                                "(kt p) s -> p kt s", p=128
                            ),
                        )
                        xts[(pn, h2)] = x_t

                # chunk-interleaved Q/K so attention on chunk 0 starts early
                for ch in range(4):
                    for pn, w_sb, dst, bias in (
                        ("q", w_q, qt_sb, bq_sb),
                        ("k", w_k, kt_sb, bk_sb),
                    ):
                        for sh in range(2):
                            ps = pjps.tile(
                                [128, 512], F32, tag="pjps",
                                name=f"pj_{pn}_{ch}{sh}",
                            )
                            for kt in range(8):
                                nc.tensor.matmul(
                                    ps[:],
                                    w_sb[:, kt, ch * 128 : (ch + 1) * 128],
                                    xts[(pn, kt // 4)][:, kt % 4, sh * 512 : (sh + 1) * 512],
                                    start=(kt == 0),
                                    stop=(kt == 7),
                                )
                            nc.vector.tensor_scalar_add(
                                dst[:, ch, sh * 512 : (sh + 1) * 512],
                                ps[:],
                                bias[:, ch : ch + 1],
                            )

                # V projection, streamed per s-tile
                w_v = wtpool.tile([128, 8, ESL], BF16, tag="wt", name="w_v")
                nc.sync.dma_start(
                    w_v[:], wvt.ap().rearrange("(kt p) n -> p kt n", p=128)
                )
                xvs = []
                for h2 in range(2):
                    x_t = xtpool.tile([128, 4, S], BF16, tag="xt", name=f"x_v{h2}")
                    nc.sync.dma_start(
                        x_t[:],
                        xtv.ap()[h2 * 512 : (h2 + 1) * 512, :].rearrange(
                            "(kt p) s -> p kt s", p=128
                        ),
                    )
                    xvs.append(x_t)
                for st in range(8):
                    ps = pjps.tile([128, 512], F32, tag="pjps", name=f"pj_v_{st}")
                    for kt in range(8):
                        nc.tensor.matmul(
                            ps[:],
                            xvs[kt // 4][:, kt % 4, st * 128 : (st + 1) * 128],
                            w_v[:, kt, :],
                            start=(kt == 0),
                            stop=(kt == 7),
                        )
                    nc.vector.tensor_copy(
                        v_sb[:, st, :, 0:HD],
                        ps[:].rearrange("p (h e) -> p h e", e=HD),
                    )

            # ---- Phase 2: attention, single pass per head ----
            ct_sb = qkvpool.tile([128, 4, S], BF16)
            wo_sb = wtpool.tile([128, 4, EMB], BF16, tag="wt")
            nc.sync.dma_start(
                wo_sb[:], wot.ap().rearrange("(ce p) n -> p ce n", p=128)
            )

            with (
                tc.tile_pool(name="stps", bufs=2, space="PSUM") as stpsum,
                tc.tile_pool(name="pvps", bufs=4, space="PSUM") as pvpsum,
                tc.tile_pool(name="et", bufs=4) as etpool,
                tc.tile_pool(name="small", bufs=3) as smallpool,
                tc.tile_pool(name="bc", bufs=2) as bcpool,
            ):
                # The PE queue is strictly in-order: a PV matmul waiting on
                # its exp would block later (independent) ST matmuls queued
                # behind it. Emit ST two steps ahead of exp/PV so the PE
                # always has ready work in front of any waiting instruction.
                LOOKAHEAD = 1
                steps = [(h, kt) for h in range(HPC) for kt in range(8)]
                st_tiles = {}

                def emit_st(idx):
                    h, kt = steps[idx]
                    hb = 64 * (h % 2)
                    hc = h // 2
                    st_ps = stpsum.tile(
                        [128, 1024], F32, tag="stps", name=f"st_{h}_{kt}"
                    )
                    for qh in range(2):
                        nc.tensor.matmul(
                            st_ps[:, qh * 512 : (qh + 1) * 512],
                            kt_sb[hb : hb + 64, hc, kt * 128 : (kt + 1) * 128],
                            qt_sb[hb : hb + 64, hc, qh * 512 : (qh + 1) * 512],
                            start=True,
                            stop=True,
                            tile_position=(hb, 0),
                        )
                    st_tiles[idx] = st_ps

                def _finish_head(h):
                    hc = h // 2
                    pv0, pv1 = pvs[h]
                    # reciprocal denominators: DVE hop PSUM row 64 -> SBUF
                    # (lanes can't shift partitions), then one DMA splits the
                    # two q-halves onto partitions 0/1
                    sums_row = smallpool.tile(
                        [65, 1024], F32, tag="sumsrow", name=f"sr{h}"
                    )
                    nc.vector.tensor_copy(sums_row[64:65, 0:512], pv0[64:65, :])
                    nc.vector.tensor_copy(sums_row[64:65, 512:1024], pv1[64:65, :])
                    sums2 = smallpool.tile([2, 512], F32, tag="sums2", name=f"sm{h}")
                    nc.gpsimd.dma_start(sums2[:], sums_row[64:65, :])
                    recip2 = smallpool.tile([2, 512], F32, tag="recip2", name=f"rc{h}")
                    nc.vector.reciprocal(recip2[:], sums2[:])
                    nc.sync.dma_start(rsum_d.ap()[h : h + 1, :], recip2[:, :])
                    bc_sb = bcpool.tile([64, 1024], F32, tag="bc", name=f"bc{h}")
                    for qh in range(2):
                        nc.sync.dma_start(
                            bc_sb[:, qh * 512 : (qh + 1) * 512],
                            recip2[qh : qh + 1, :]
                            .unsqueeze(1)
                            .to_broadcast([1, 64, 512]),
                        )
                    # matmul dst starts at partition 0; odd heads hop to
                    # partitions 64-127 of ct_sb via a small SBUF DMA
                    if h % 2 == 0:
                        for qh, pv in ((0, pv0), (1, pv1)):
                            nc.vector.tensor_tensor(
                                ct_sb[0:64, hc, qh * 512 : (qh + 1) * 512],
                                pv[0:64, :],
                                bc_sb[0:64, qh * 512 : (qh + 1) * 512],
                                op=MULT,
                            )
                    else:
                        ct_tmp = bcpool.tile(
                            [64, 1024], BF16, tag="cttmp", name=f"ctt{h}"
                        )
                        for qh, pv in ((0, pv0), (1, pv1)):
                            nc.vector.tensor_tensor(
                                ct_tmp[:, qh * 512 : (qh + 1) * 512],
                                pv[0:64, :],
                                bc_sb[0:64, qh * 512 : (qh + 1) * 512],
                                op=MULT,
                            )
                        nc.sync.dma_start(ct_sb[64:128, hc, :], ct_tmp[:])

                pvs = {}
                for h in range(HPC):
                    # rows 0-63: context^T; row 64: softmax denominator
                    pvs[h] = [
                        pvpsum.tile([65, 512], F32, tag="pv", name=f"pv{h}_{qh}")
                        for qh in range(2)
                    ]
                    for kt in range(8):
                        idx = h * 8 + kt
                        if idx == 0:
                            for j in range(LOOKAHEAD + 1):
                                emit_st(j)
                        elif idx + LOOKAHEAD < len(steps):
                            emit_st(idx + LOOKAHEAD)
                        st_ps = st_tiles.pop(idx)
                        et_t = etpool.tile([128, 1024], F32R, tag="et")
                        nc.scalar.activation(et_t[:], st_ps[:], EXP, scale=SCALE)
                        # unnormalized weights out (host normalizes+transposes)
                        nc.gpsimd.dma_start(
                            wtsu_d.ap()[h, kt * 128 : (kt + 1) * 128, :], et_t[:]
                        )
                        for qh, pv in ((0, pvs[h][0]), (1, pvs[h][1])):
                            nc.tensor.matmul(
                                pv[:],
                                v_sb[:, kt, h, :],
                                et_t[:, qh * 512 : (qh + 1) * 512],
                                start=(kt == 0),
                                stop=(kt == 7),
                            )
                    _finish_head(h)

            # ---- Phase 3: output projection (partial; host sums core pairs) ----
            with (
                tc.tile_pool(name="ops", bufs=2, space="PSUM") as oppsum,
                tc.tile_pool(name="osb", bufs=2) as outpool,
            ):
                for ch in range(8):
                    o_sb = outpool.tile([128, 1024], F32, tag="osb")
                    for sh in range(2):
                        o_ps = oppsum.tile([128, 512], F32, tag="ops")
                        for ce in range(4):
                            nc.tensor.matmul(
                                o_ps[:],
                                wo_sb[:, ce, ch * 128 : (ch + 1) * 128],
                                ct_sb[:, ce, sh * 512 : (sh + 1) * 512],
                                start=(ce == 0),
                                stop=(ce == 3),
                            )
                        nc.vector.tensor_scalar_add(
                            o_sb[:, sh * 512 : (sh + 1) * 512],
                            o_ps[:],
                            bo_sb[:, ch : ch + 1],
                        )
                    nc.sync.dma_start(
                        outp_d.ap()[ch * 128 : (ch + 1) * 128, :], o_sb[:]
                    )

    nc.compile()
    _CACHE["nc"] = nc
    return nc


def kernel(**inputs):
    query = np.asarray(inputs["query"], np.float32)
    key = np.asarray(inputs["key"], np.float32)
    value = np.asarray(inputs["value"], np.float32)
    Wq, bq = np.asarray(inputs["Wq"], np.float32), np.asarray(inputs["bq"], np.float32)
    Wk, bk = np.asarray(inputs["Wk"], np.float32), np.asarray(inputs["bk"], np.float32)
    Wv, bv = np.asarray(inputs["Wv"], np.float32), np.asarray(inputs["bv"], np.float32)
    Wo, bo = np.asarray(inputs["Wo"], np.float32), np.asarray(inputs["bo"], np.float32)

    nc = _build()

    in_maps = []
    for c in range(NCORES):
        b, g = divmod(c, 2)
        cols = slice(g * ESL, (g + 1) * ESL)
        # bv folds through the (linear) output projection: W @ (V + bv) adds
        # Wo_c^T @ bv_c per core; bo itself is added by the even core only.
        bo_eff = Wo.T[cols, :].T @ bv[cols]
        if g == 0:
            bo_eff = bo_eff + bo
        in_maps.append(
            {
                "xtq": np.ascontiguousarray(query[b].T).astype(BF16_NP),
                "xtk": np.ascontiguousarray(key[b].T).astype(BF16_NP),
                "xtv": np.ascontiguousarray(value[b].T).astype(BF16_NP),
                "wqt": np.ascontiguousarray(Wq.T[:, cols]).astype(BF16_NP),
                "wkt": np.ascontiguousarray(Wk.T[:, cols]).astype(BF16_NP),
                "wvt": np.ascontiguousarray(Wv.T[:, cols]).astype(BF16_NP),
                "wot": np.ascontiguousarray(Wo.T[cols, :]).astype(BF16_NP),
                "bq": np.ascontiguousarray(bq[cols].reshape(4, 128).T),
                "bk": np.ascontiguousarray(bk[cols].reshape(4, 128).T),
                "bo": np.ascontiguousarray(bo_eff.reshape(8, 128).T),
                "vones": np.ones((128, 8, HPC), np.float32),
            }
        )

    res = run_bass_kernel_spmd(nc, in_maps, list(range(NCORES)))

    out = np.empty((B, S, EMB), np.float32)
    wts = np.empty((B, HEADS, S, S), np.float32)
    for c in range(NCORES):
        b, g = divmod(c, 2)
        # normalize + [k,q]->[q,k] transpose fused on host
        wu = res.results[c]["wtsu"]  # [8, k, q] unnormalized exp
        rs = res.results[c]["rsum"]  # [8, q] reciprocal row sums
        np.einsum(
            "hkq,hq->hqk", wu, rs, out=wts[b, g * HPC : (g + 1) * HPC],
            optimize=False,
        )
    for b in range(B):
        out[b] = (res.results[2 * b]["outp"] + res.results[2 * b + 1]["outp"]).T
    return out, wts
